# revision 17
# baseline (speedup 1.0000x reference)
"""Dense-MoE (top-2 of 8 experts) TRN2 kernel: expert-parallel over 8 NeuronCores.

Host side: softmax + top-2 routing, per-expert token gather (padded to the max
expert load), weight re-layout. Device side (per core = one expert):
    h = silu(x_e @ gw.T) * (x_e @ uw.T)        [F-major in SBUF]
    out_e = (h @ dw.T) * w_token                [rows scaled by routing weight]
Host scatter-adds the 8 per-expert outputs into the [T, D] result.

All matmuls run in float32r (tf32-like, ~1e-4 rel err, full PE throughput).
"""
import sys

sys.path.insert(0, "/opt/trn_rl_repo")

import numpy as np

import concourse.bass as bass
from concourse import bacc
import concourse.mybir as mybir
import concourse.tile as tile
from concourse.bass_utils import run_bass_kernel_spmd
from concourse.bass import ds

T, D, F, E, TOPK = 4096, 1024, 2048, 8, 2
P = 128
N_CORES = 8

F32 = mybir.dt.float32
F32R = mybir.dt.float32r


def _build(cap):
    """Build the per-core Bass program for capacity `cap` tokens (cap % 128 == 0)."""
    n_ct = cap // P          # token tiles of 128
    d_out_tiles = D // 512   # 2
    # Token chunks for the gate/up phase: as uniform as possible in units of
    # 128, each <=512 (PSUM bank limit). Uniform sizes avoid a trailing
    # LDW-bound narrow chunk.
    nch = -(-cap // 512)
    base = (cap // nch) // P * P
    sizes = [base] * nch
    rem = (cap - base * nch) // P
    for i in range(rem):
        sizes[-1 - i] += P
    sizes.sort()  # smallest chunk first: the opening psum group needs less x
    chunks = []
    c0 = 0
    for cs in sizes:
        chunks.append((c0, cs))
        c0 += cs

    nc = bacc.Bacc(None, target_bir_lowering=False)
    x_d = nc.declare_dram_parameter("x", [P, D // P, cap], F32R, isOutput=False)
    gw_d = nc.declare_dram_parameter("gw", [P, F // P, D // P, P], F32R, isOutput=False)
    uw_d = nc.declare_dram_parameter("uw", [P, F // P, D // P, P], F32R, isOutput=False)
    dw_d = nc.declare_dram_parameter("dw", [P, F // P, D], F32R, isOutput=False)
    tw_d = nc.declare_dram_parameter("tw", [P, n_ct], F32, isOutput=False)
    out_d = nc.declare_dram_parameter("out", [cap, D], F32, isOutput=True)

    with tile.TileContext(nc) as tc:
        with (
            tc.tile_pool(name="deep", bufs=1) as deep,
            tc.tile_pool(name="wts", bufs=3) as wts,
            tc.tile_pool(name="stage", bufs=2) as stage,
            tc.tile_pool(name="ps", bufs=2, space="PSUM") as ps,
        ):
            # First gate/up weight tiles go ahead of everything so the PE can
            # start as early as possible; x is split per d-slice for the same
            # reason (the first psum group consumes slices progressively).
            wt_tiles = {}

            def load_ft(ft):
                gw_t = wts.tile([P, D // P, P], F32R, tag="gw")
                nc.sync.dma_start(gw_t[:], gw_d[:, ft])
                uw_t = wts.tile([P, D // P, P], F32R, tag="uw")
                nc.sync.dma_start(uw_t[:], uw_d[:, ft])
                wt_tiles[ft] = (gw_t, uw_t)

            # HAM warm-up: dependency-free matmuls on a memset tile bring the
            # PE clock to 2.4 GHz while the first real operands are in flight.
            warm32 = stage.tile([P, P], F32, tag="warm32")
            nc.vector.memset(warm32[:], 0.0)
            warm = stage.tile([P, P], F32R, tag="warm")
            nc.vector.tensor_copy(warm[:], warm32[:])
            pw = ps.tile([P, P], F32, tag="pw")
            for i in range(24):
                nc.tensor.matmul(pw[:], warm[:], warm[:],
                                 start=(i == 0), stop=(i == 23))

            load_ft(0)
            x_t = deep.tile([P, D // P, cap], F32R, tag="x")
            c00, cs0 = chunks[0]
            for dt_ in range(D // P):
                nc.sync.dma_start(x_t[:, dt_, ds(c00, cs0)], x_d[:, dt_, ds(c00, cs0)])
            for dt_ in range(D // P):
                nc.sync.dma_start(x_t[:, dt_, ds(cs0, cap - cs0)],
                                  x_d[:, dt_, ds(cs0, cap - cs0)])
            tw_t = deep.tile([P, n_ct], F32, tag="tw")
            nc.sync.dma_start(tw_t[:], tw_d[:])
            h_t = deep.tile([P, F // P, cap], F32R, tag="h")
            dw_t = deep.tile([P, F // P, D], F32R, tag="dw")

            # Phase A: h[fp, ft, c] = silu(g) * u, F-major
            for ft in range(F // P):
                if ft == 8:
                    # Down weights, slice-by-slice, enqueued once phase A's own
                    # traffic has drained; ready well before phase B needs them.
                    for fo in range(F // P):
                        nc.sync.dma_start(dw_t[:, fo], dw_d[:, fo])
                if ft not in wt_tiles:
                    load_ft(ft)
                gw_t, uw_t = wt_tiles.pop(ft)
                for (c0, cs) in chunks:
                    pg = ps.tile([P, 512], F32, tag="pg")
                    for dt_ in range(D // P):
                        nc.tensor.matmul(
                            pg[:, :cs], gw_t[:, dt_], x_t[:, dt_, ds(c0, cs)],
                            start=(dt_ == 0), stop=(dt_ == D // P - 1),
                        )
                    pu = ps.tile([P, 512], F32, tag="pu")
                    for dt_ in range(D // P):
                        nc.tensor.matmul(
                            pu[:, :cs], uw_t[:, dt_], x_t[:, dt_, ds(c0, cs)],
                            start=(dt_ == 0), stop=(dt_ == D // P - 1),
                        )
                    sg = stage.tile([P, 512], F32, tag="sg")
                    nc.scalar.activation(sg[:, :cs], pg[:, :cs],
                                         mybir.ActivationFunctionType.Silu)
                    nc.vector.tensor_tensor(
                        h_t[:, ft, ds(c0, cs)], sg[:, :cs], pu[:, :cs],
                        mybir.AluOpType.mult,
                    )

            # Phase B: out[c, d] = sum_f h[f, c] * dwT[f, d], scaled by tw
            for do in range(d_out_tiles):
                for ct in range(n_ct):
                    po = ps.tile([P, 512], F32, tag="po")
                    for fo in range(F // P):
                        nc.tensor.matmul(
                            po[:], h_t[:, fo, ds(ct * P, P)], dw_t[:, fo, ds(do * 512, 512)],
                            start=(fo == 0), stop=(fo == F // P - 1),
                        )
                    osb = stage.tile([P, 512], F32, tag="osb")
                    nc.scalar.activation(osb[:], po[:],
                                         mybir.ActivationFunctionType.Copy,
                                         scale=tw_t[:, ds(ct, 1)])
                    nc.sync.dma_start(out_d[ds(ct * P, P), ds(do * 512, 512)], osb[:])
    nc.finalize()
    return nc


def _route(gating_output):
    """Numpy softmax + top-2 + renormalize; returns (ids [T,K], w [T,K])."""
    g = gating_output.astype(np.float32)
    m = g.max(axis=-1, keepdims=True)
    e = np.exp(g - m)
    probs = e / e.sum(axis=-1, keepdims=True)
    ids = np.argsort(-probs, axis=-1, kind="stable")[:, :TOPK]
    w = np.take_along_axis(probs, ids, axis=-1)
    w = w / w.sum(axis=-1, keepdims=True)
    return ids, w


def kernel(x, gating_output, gate_w, up_w, down_w):
    x = np.asarray(x, dtype=np.float32)
    gating_output = np.asarray(gating_output, dtype=np.float32)
    gate_w = np.asarray(gate_w, dtype=np.float32)
    up_w = np.asarray(up_w, dtype=np.float32)
    down_w = np.asarray(down_w, dtype=np.float32)

    ids, w = _route(gating_output)

    # Token lists per expert
    idx_e = []
    w_e = []
    for e in range(E):
        sel = np.nonzero((ids == e).any(axis=-1))[0]
        kpos = (ids[sel] == e).argmax(axis=-1)
        idx_e.append(sel)
        w_e.append(w[sel, kpos])

    cap = max(len(i) for i in idx_e)
    cap = ((cap + P - 1) // P) * P
    n_ct = cap // P

    nc = _build(cap)

    in_maps = []
    for e in range(E):
        idx = idx_e[e]
        cnt = len(idx)
        x_pad = np.zeros((cap, D), dtype=np.float32)
        x_pad[:cnt] = x[idx]
        tw_pad = np.zeros((cap,), dtype=np.float32)
        tw_pad[:cnt] = w_e[e]

        # x: [cap, D] -> [128(dp), D/128(do), cap]
        x_dev = np.ascontiguousarray(
            x_pad.T.reshape(D // P, P, cap).transpose(1, 0, 2))
        # gate/up: [F, D] -> T -> [D, F] -> [128(dp), 16(ft), 8(do), 128(fi)]
        gwT = gate_w[e].T  # [D, F]
        gw_dev = np.ascontiguousarray(
            gwT.reshape(D // P, P, F // P, P).transpose(1, 2, 0, 3))
        uwT = up_w[e].T
        uw_dev = np.ascontiguousarray(
            uwT.reshape(D // P, P, F // P, P).transpose(1, 2, 0, 3))
        # down: [D, F] -> T -> [F, D] -> [128(fp), 16(fo), D]
        dwT = down_w[e].T  # [F, D]
        dw_dev = np.ascontiguousarray(
            dwT.reshape(F // P, P, D).transpose(1, 0, 2))
        tw_dev = np.ascontiguousarray(tw_pad.reshape(n_ct, P).T)

        in_maps.append({
            "x": x_dev, "gw": gw_dev, "uw": uw_dev, "dw": dw_dev, "tw": tw_dev,
        })

    try:
        res = run_bass_kernel_spmd(nc, in_maps, core_ids=list(range(N_CORES)))
    except Exception:
        # First execution of a fresh NEFF occasionally dies with
        # NRT_EXEC_UNIT_UNRECOVERABLE on this setup; the retry reuses the
        # cached executable and goes through.
        import time as _time

        _time.sleep(5)
        res = run_bass_kernel_spmd(nc, in_maps, core_ids=list(range(N_CORES)))

    out = np.zeros((T, D), dtype=np.float32)
    for e in range(E):
        cnt = len(idx_e[e])
        out[idx_e[e]] += res.results[e]["out"][:cnt]
    return out


# revision 24
# speedup vs baseline: 1.0438x; 1.0438x over previous
"""Dense-MoE (top-2 of 8 experts) TRN2 kernel: expert-parallel over 8 NeuronCores.

Host side: softmax + top-2 routing, per-expert token gather (padded to the max
expert load), weight re-layout. Device side (per core = one expert):
    h = silu(x_e @ gw.T) * (x_e @ uw.T)        [F-major in SBUF]
    out_e = (h @ dw.T) * w_token                [rows scaled by routing weight]
Host scatter-adds the 8 per-expert outputs into the [T, D] result.

All matmuls run in float32r (tf32-like, ~1e-4 rel err, full PE throughput).
"""
import sys

sys.path.insert(0, "/opt/trn_rl_repo")

import numpy as np

import concourse.bass as bass
from concourse import bacc
import concourse.mybir as mybir
import concourse.tile as tile
from concourse.bass_utils import run_bass_kernel_spmd
from concourse.bass import ds

T, D, F, E, TOPK = 4096, 1024, 2048, 8, 2
P = 128
N_CORES = 8

F32 = mybir.dt.float32
F32R = mybir.dt.float32r


def _build(cap):
    """Build the per-core Bass program for capacity `cap` tokens (= max expert
    load, no rounding: matmul free dims take any size <=512, and the trailing
    partial token tile only occupies cap%128 PSUM partitions)."""
    assert cap % 2 == 0  # fp32r matmul free dims must be even
    n_ct = -(-cap // P)      # token tiles of <=128 (last may be partial)
    d_out_tiles = D // 512   # 2
    # Token chunks for the gate/up phase: uniform even sizes, each <=512
    # (PSUM bank cap; fp32r needs even free dims)
    nch = -(-cap // 512)
    base = (cap // nch) & ~1
    sizes = [base] * nch
    rem = cap - base * nch
    i = 0
    while rem > 0:
        sizes[-1 - (i % nch)] += 2
        rem -= 2
        i += 1
    chunks = []
    c0 = 0
    for cs in sizes:
        chunks.append((c0, cs))
        c0 += cs

    nc = bacc.Bacc(None, target_bir_lowering=False)
    x_d = nc.declare_dram_parameter("x", [P, D // P, cap], F32R, isOutput=False)
    gw_d = nc.declare_dram_parameter("gw", [P, F // P, D // P, P], F32R, isOutput=False)
    uw_d = nc.declare_dram_parameter("uw", [P, F // P, D // P, P], F32R, isOutput=False)
    dw_d = nc.declare_dram_parameter("dw", [P, F // P, D], F32R, isOutput=False)
    tw_d = nc.declare_dram_parameter("tw", [P, n_ct], F32, isOutput=False)
    out_d = nc.declare_dram_parameter("out", [cap, D], F32, isOutput=True)

    with tile.TileContext(nc) as tc:
        with (
            tc.tile_pool(name="deep", bufs=1) as deep,
            tc.tile_pool(name="wts", bufs=3) as wts,
            tc.tile_pool(name="stage", bufs=2) as stage,
            tc.tile_pool(name="ps", bufs=2, space="PSUM") as ps,
        ):
            # First gate/up weight tiles go ahead of everything so the PE can
            # start as early as possible; x is split per d-slice for the same
            # reason (the first psum group consumes slices progressively).
            wt_tiles = {}

            def load_ft(ft):
                gw_t = wts.tile([P, D // P, P], F32R, tag="gw")
                nc.sync.dma_start(gw_t[:], gw_d[:, ft])
                uw_t = wts.tile([P, D // P, P], F32R, tag="uw")
                nc.sync.dma_start(uw_t[:], uw_d[:, ft])
                wt_tiles[ft] = (gw_t, uw_t)

            load_ft(0)
            x_t = deep.tile([P, D // P, cap], F32R, tag="x")
            c00, cs0 = chunks[0]
            for dt_ in range(D // P):
                nc.sync.dma_start(x_t[:, dt_, ds(c00, cs0)], x_d[:, dt_, ds(c00, cs0)])
            for dt_ in range(D // P):
                nc.sync.dma_start(x_t[:, dt_, ds(cs0, cap - cs0)],
                                  x_d[:, dt_, ds(cs0, cap - cs0)])
            tw_t = deep.tile([P, n_ct], F32, tag="tw")
            nc.sync.dma_start(tw_t[:], tw_d[:])
            h_t = deep.tile([P, F // P, cap], F32R, tag="h")
            dw_t = deep.tile([P, F // P, D], F32R, tag="dw")

            # Phase A: h[fp, ft, c] = silu(g) * u, F-major
            for ft in range(F // P):
                if ft == 8:
                    # Down weights, slice-by-slice, enqueued once phase A's own
                    # traffic has drained; ready well before phase B needs them.
                    for fo in range(F // P):
                        nc.sync.dma_start(dw_t[:, fo], dw_d[:, fo])
                if ft not in wt_tiles:
                    load_ft(ft)
                gw_t, uw_t = wt_tiles.pop(ft)
                for (c0, cs) in chunks:
                    pg = ps.tile([P, 512], F32, tag="pg")
                    for dt_ in range(D // P):
                        nc.tensor.matmul(
                            pg[:, :cs], gw_t[:, dt_], x_t[:, dt_, ds(c0, cs)],
                            start=(dt_ == 0), stop=(dt_ == D // P - 1),
                        )
                    pu = ps.tile([P, 512], F32, tag="pu")
                    for dt_ in range(D // P):
                        nc.tensor.matmul(
                            pu[:, :cs], uw_t[:, dt_], x_t[:, dt_, ds(c0, cs)],
                            start=(dt_ == 0), stop=(dt_ == D // P - 1),
                        )
                    sg = stage.tile([P, 512], F32, tag="sg")
                    nc.scalar.activation(sg[:, :cs], pg[:, :cs],
                                         mybir.ActivationFunctionType.Silu)
                    nc.vector.tensor_tensor(
                        h_t[:, ft, ds(c0, cs)], sg[:, :cs], pu[:, :cs],
                        mybir.AluOpType.mult,
                    )

            # Phase B: out[c, d] = sum_f h[f, c] * dwT[f, d], scaled by tw
            for do in range(d_out_tiles):
                for ct in range(n_ct):
                    csz = min(P, cap - ct * P)
                    po = ps.tile([P, 512], F32, tag="po")
                    for fo in range(F // P):
                        nc.tensor.matmul(
                            po[:csz], h_t[:, fo, ds(ct * P, csz)],
                            dw_t[:, fo, ds(do * 512, 512)],
                            start=(fo == 0), stop=(fo == F // P - 1),
                        )
                    osb = stage.tile([P, 512], F32, tag="osb")
                    nc.scalar.activation(osb[:csz], po[:csz],
                                         mybir.ActivationFunctionType.Copy,
                                         scale=tw_t[:csz, ds(ct, 1)])
                    nc.sync.dma_start(out_d[ds(ct * P, csz), ds(do * 512, 512)], osb[:csz])
    nc.finalize()
    return nc


def _route(gating_output):
    """Numpy softmax + top-2 + renormalize; returns (ids [T,K], w [T,K])."""
    g = gating_output.astype(np.float32)
    m = g.max(axis=-1, keepdims=True)
    e = np.exp(g - m)
    probs = e / e.sum(axis=-1, keepdims=True)
    ids = np.argsort(-probs, axis=-1, kind="stable")[:, :TOPK]
    w = np.take_along_axis(probs, ids, axis=-1)
    w = w / w.sum(axis=-1, keepdims=True)
    return ids, w


def kernel(x, gating_output, gate_w, up_w, down_w):
    x = np.asarray(x, dtype=np.float32)
    gating_output = np.asarray(gating_output, dtype=np.float32)
    gate_w = np.asarray(gate_w, dtype=np.float32)
    up_w = np.asarray(up_w, dtype=np.float32)
    down_w = np.asarray(down_w, dtype=np.float32)

    ids, w = _route(gating_output)

    # Token lists per expert
    idx_e = []
    w_e = []
    for e in range(E):
        sel = np.nonzero((ids == e).any(axis=-1))[0]
        kpos = (ids[sel] == e).argmax(axis=-1)
        idx_e.append(sel)
        w_e.append(w[sel, kpos])

    cap = max(len(i) for i in idx_e)
    cap += cap & 1  # fp32r matmuls need even free dims
    n_ct = -(-cap // P)

    nc = _build(cap)

    in_maps = []
    for e in range(E):
        idx = idx_e[e]
        cnt = len(idx)
        x_pad = np.zeros((cap, D), dtype=np.float32)
        x_pad[:cnt] = x[idx]
        tw_pad = np.zeros((cap,), dtype=np.float32)
        tw_pad[:cnt] = w_e[e]

        # x: [cap, D] -> [128(dp), D/128(do), cap]
        x_dev = np.ascontiguousarray(
            x_pad.T.reshape(D // P, P, cap).transpose(1, 0, 2))
        # gate/up: [F, D] -> T -> [D, F] -> [128(dp), 16(ft), 8(do), 128(fi)]
        gwT = gate_w[e].T  # [D, F]
        gw_dev = np.ascontiguousarray(
            gwT.reshape(D // P, P, F // P, P).transpose(1, 2, 0, 3))
        uwT = up_w[e].T
        uw_dev = np.ascontiguousarray(
            uwT.reshape(D // P, P, F // P, P).transpose(1, 2, 0, 3))
        # down: [D, F] -> T -> [F, D] -> [128(fp), 16(fo), D]
        dwT = down_w[e].T  # [F, D]
        dw_dev = np.ascontiguousarray(
            dwT.reshape(F // P, P, D).transpose(1, 0, 2))
        tw_full = np.zeros((n_ct * P,), dtype=np.float32)
        tw_full[:cap] = tw_pad
        tw_dev = np.ascontiguousarray(tw_full.reshape(n_ct, P).T)

        in_maps.append({
            "x": x_dev, "gw": gw_dev, "uw": uw_dev, "dw": dw_dev, "tw": tw_dev,
        })

    try:
        res = run_bass_kernel_spmd(nc, in_maps, core_ids=list(range(N_CORES)))
    except Exception:
        # First execution of a fresh NEFF occasionally dies with
        # NRT_EXEC_UNIT_UNRECOVERABLE on this setup; the retry reuses the
        # cached executable and goes through.
        import time as _time

        _time.sleep(5)
        res = run_bass_kernel_spmd(nc, in_maps, core_ids=list(range(N_CORES)))

    out = np.zeros((T, D), dtype=np.float32)
    for e in range(E):
        cnt = len(idx_e[e])
        out[idx_e[e]] += res.results[e]["out"][:cnt]
    return out
